# revision 34
# baseline (speedup 1.0000x reference)
"""TRN2 Bass kernel for nn_DynamicWeightProjection.

Computes, for x = query_vec reshaped [B*T, D]:
    h   = gelu_exact(x @ W1)            W1 = dw1[:, 0, {0,2}, :]   -> 256 cols
    w_c = h_c @ qkw_c                   qkw_c = qkw[0, c] reshaped [128, 128]
    out = concat(rms(w_pre[:2]), rms(w_pre[2:])*s, tanh(x@dd)[0:32],
                 rms(w_post[:2]), rms(w_post[2:])*s, tanh(x@dd)[64:96])
Only C-splits {0, 2} and dd columns {0:32, 64:96} survive into the output,
so the fused first matmul needs just 320 of the 640 columns.

Strategy: 8-way data parallel over rows (B*T = 16384 -> 2048 rows/core).
The host pre-transposes x (contraction D on SBUF partitions, chunk-major
so each 512-row chunk is one contiguous 4.2MB DMA with 32KB descriptors).
Everything runs in bf16 (gate is rel<2e-2; bf16 end-to-end measures
~5e-3), halving HBM traffic vs fp32. The 64 tanh(x@dd) columns are
written column-major to a separate output and spliced on the host, which
removes all PE transposes. Dummy matmuls on zeroed SBUF warm the PE HAM
clock-gate during the DMA prologue. Inputs ride the sync-engine DGE ring,
outputs the scalar ring, so stores never head-of-line-block loads.
Postprocessing (mm2 + rms + pack) is batched per row-block pair and
software-pipelined one chunk behind mm1.
"""
import numpy as np
from contextlib import ExitStack

import ml_dtypes

import concourse.bacc as bacc
import concourse.mybir as mybir
import concourse.tile as tile
from concourse.bass_utils import run_bass_kernel_spmd

AF = mybir.ActivationFunctionType
F32 = mybir.dt.float32
BF16 = mybir.dt.bfloat16

B, T, D = 4, 4096, 4096
NCORES = 8
ROWS = (B * T) // NCORES        # 2048 rows per core
RC = 512                        # rows per row-chunk (PSUM accumulation unit)
NRC = ROWS // RC                # 4 chunks
DC = D // 128                   # 32 contraction chunks
WCOLS = 320                     # 256 w-cols (c=0,2) + 32 dd_pre + 32 dd_post
EPS = 1.1920929e-07


def build_nc(nrc=NRC, rc=RC, s2_scale=31250.0, s2_bias=EPS * 1e6, act=None,
             repeat=1):
    """Build the per-core SPMD program. s2_scale/s2_bias fold norm_scale into
    the w2 rms factor: rms(v)*s == 1/sqrt(ssum/(32 s^2) + eps/s^2)."""
    if act is None:
        act = AF.Gelu
    nc = bacc.Bacc("TRN2", target_bir_lowering=False, debug=False,
                   num_devices=NCORES, enable_partition_id=False)
    rows = nrc * rc

    xt_in = nc.dram_tensor("xt", [128, nrc, DC, rc], BF16, kind="ExternalInput")
    wall_in = nc.dram_tensor("wall", [128, DC, WCOLS], BF16, kind="ExternalInput")
    qkw_in = nc.dram_tensor("qkw2", [128, 2, 128], BF16, kind="ExternalInput")
    mrg_in = nc.dram_tensor("mrg", [128, 64], BF16, kind="ExternalInput")
    out_d = nc.dram_tensor("out", [rows, 256], F32, kind="ExternalOutput")
    # dd (tanh) output kept column-major [64, rows]; host transposes/splices
    dd_d = nc.dram_tensor("ddo", [64, rows], F32, kind="ExternalOutput")

    with tile.TileContext(nc) as tc, ExitStack() as ctx:
        consts = ctx.enter_context(tc.tile_pool(name="consts", bufs=1))
        xpool = ctx.enter_context(tc.tile_pool(name="x", bufs=4))
        hpool = ctx.enter_context(tc.tile_pool(name="h", bufs=2))
        wpool = ctx.enter_context(tc.tile_pool(name="w", bufs=3))
        spool = ctx.enter_context(tc.tile_pool(name="s", bufs=4))
        papool = ctx.enter_context(tc.tile_pool(name="pack", bufs=2))
        ph = ctx.enter_context(tc.tile_pool(name="ph", bufs=2, space="PSUM"))
        po = ctx.enter_context(tc.tile_pool(name="po", bufs=2, space="PSUM"))

        wall_sb = consts.tile([128, DC, WCOLS], BF16)
        qkw_sb = consts.tile([128, 2, 128], BF16)
        mrg_sb = consts.tile([128, 64], BF16)
        wz = consts.tile([128, RC], BF16)
        bias1 = consts.tile([128, 1], F32)
        bias2 = consts.tile([128, 1], F32)

        it_list = [(rep, ci) for rep in range(repeat) for ci in range(nrc)]
        xtiles = {}

        def ensure_load(idx):
            if idx >= len(it_list) or idx in xtiles:
                return
            ci = it_list[idx][1]
            t = xpool.tile([128, DC, rc], BF16, tag="xt")
            # 4 sub-DMAs per chunk (8KB/partition each) so mm1 can start on
            # the first 8 d-chunks instead of waiting for the whole 4.2MB
            for g0 in range(0, DC, 8):
                nc.sync.dma_start(t[:, g0:g0 + 8], xt_in[:, ci, g0:g0 + 8])
            xtiles[idx] = t

        # Prologue: interleave the first chunk's x slices with the weight
        # stream in PE consumption order; prefetch the remaining chunks.
        xt0 = xpool.tile([128, DC, rc], BF16, tag="xt")
        for g0, g1 in ((0, 2), (2, 8), (8, 14), (14, 20), (20, 26), (26, DC)):
            nc.sync.dma_start(wall_sb[:, g0:g1, :], wall_in[:, g0:g1, :])
            nc.sync.dma_start(xt0[:, g0:g1], xt_in[:, 0, g0:g1])
        nc.sync.dma_start(qkw_sb[:], qkw_in[:])
        nc.sync.dma_start(mrg_sb[:], mrg_in[:])
        xtiles[0] = xt0
        for idx in range(1, min(nrc, len(it_list))):
            ensure_load(idx)

        nc.vector.memset(wz[:], 0.0)
        nc.vector.memset(bias1[:], EPS)
        nc.vector.memset(bias2[:], s2_bias)

        # HAM warmup: ~9 cold matmuls (~3.8us > the 3.41us HAM window) on
        # zeros while the first DMAs stream in, so the real chain starts at
        # 2.4GHz. Uses the same 128x32 col-tiling mode as the real matmuls
        # (a mode switch would drain the PE).
        warm_ps = po.tile([128, 2, 256], F32, tag="w")
        wv = warm_ps[:].rearrange("p a b -> p (a b)")
        for _ in range(9):
            nc.tensor.matmul(wv[0:32, :], wz[:, 0:32], wz[:],
                             start=True, stop=True, tile_position=(0, 0))

        def emit_post_pair(pk, p, hT0, hT1):
            """mm2 + rms + pack for one row-block pair of a chunk."""
            wps = po.tile([128, 2, 256], F32, tag="w")
            for j in range(2):
                rb = 2 * p + j
                for c, hT in ((0, hT0), (1, hT1)):
                    for s in range(4):
                        nc.tensor.matmul(
                            wps[32 * s:32 * s + 32, j,
                                c * 128:(c + 1) * 128],
                            hT[:, rb * 128 + 32 * s:
                               rb * 128 + 32 * s + 32],
                            qkw_sb[:, c, :], start=True, stop=True,
                            tile_position=(0, 32 * s))
            wsb = wpool.tile([128, 2, 256], F32, tag="wsb")
            nc.scalar.activation(wsb[:], wps[:], AF.Copy)
            sq = wpool.tile([128, 2, 256], F32, tag="sq")
            nc.vector.tensor_mul(sq[:], wsb[:], wsb[:])
            ss = spool.tile([128, 16], F32, tag="ss")
            nc.vector.reduce_sum(
                ss[:].rearrange("p (a g) -> p a g", a=2),
                sq[:].rearrange("p a (g m) -> p a g m", m=32),
                axis=mybir.AxisListType.X)
            # fac = sqrt(ss*scale + bias); groups i<2 are w1 (plain
            # rms), i>=2 are w2 (norm_scale folded into scale/bias)
            fac = spool.tile([128, 16], F32, tag="fac")
            ssv = ss[:].rearrange("p (ac i) -> p ac i", i=4)
            fav = fac[:].rearrange("p (ac i) -> p ac i", i=4)
            nc.scalar.activation(fav[:, :, 0:2], ssv[:, :, 0:2],
                                 AF.Sqrt, scale=1.0 / 32.0,
                                 bias=bias1[:, 0:1])
            nc.scalar.activation(fav[:, :, 2:4], ssv[:, :, 2:4],
                                 AF.Sqrt, scale=s2_scale,
                                 bias=bias2[:, 0:1])
            rfac = spool.tile([128, 16], F32, tag="rfac")
            nc.vector.reciprocal(rfac[:], fac[:])
            rfb = rfac[:].rearrange("p (a g o) -> p a g o", a=2, o=1)
            nc.vector.tensor_mul(
                pk[:, 2 * p:2 * p + 2].rearrange(
                    "p a (g m) -> p a g m", m=32),
                wsb[:].rearrange("p a (g m) -> p a g m", m=32),
                rfb.broadcast_to([128, 2, 8, 32]))

        def store_pair(pk, row0, p):
            out_view = out_d[row0 + p * 256:row0 + (p + 1) * 256, :].rearrange(
                "(rb q) c -> q rb c", q=128)
            nc.scalar.dma_start(out_view, pk[:, 2 * p:2 * p + 2])

        def make_post(row0, hT0, hT1):
            """Full post for one chunk. Deferred into the middle of the next
            chunk's mm1 so PE's FIFO isn't blocked behind the ACT-dependent
            mm2, but its ACT/DVE chain still overlaps that chunk's mm1."""
            def post():
                pk = papool.tile([128, 4, 256], F32, tag="pk")
                emit_post_pair(pk, 0, hT0, hT1)
                store_pair(pk, row0, 0)
                emit_post_pair(pk, 1, hT0, hT1)
                store_pair(pk, row0, 1)
            return post

        pending = None
        pending_merge = None
        for idx, (rep, ci) in enumerate(it_list):
            ensure_load(idx)
            ensure_load(idx + 1)
            xt = xtiles.pop(idx)
            row0 = ci * rc

            h0 = ph.tile([128, rc], F32, tag="h0")
            h1 = ph.tile([128, rc], F32, tag="h1")
            h2 = ph.tile([128, rc], F32, tag="h2")
            hT0 = hpool.tile([128, rc], BF16, tag="hT0")
            hT1 = hpool.tile([128, rc], BF16, tag="hT1")
            h2sb = hpool.tile([128, rc], BF16, tag="h2sb")
            ddT = hpool.tile([64, rc], F32, tag="ddT")
            last = idx == len(it_list) - 1

            # mm1, column-tiled 128x32: each 32-col block of W runs on its
            # own PE col-tile so one tile's LDWEIGHTS overlaps the other
            # tiles' matmuls (this environment compiles with ldw-opt off,
            # which otherwise serializes every LDW behind the prior MM).
            # dd partials go to strips 0/1 on even dc, 2/3 on odd dc
            # (summed afterwards) so all four tiles stay equally loaded.
            def mm_h(dst, base, s, dc):
                nc.tensor.matmul(
                    dst[32 * s:32 * s + 32, :],
                    wall_sb[:, dc, base + 32 * s:base + 32 * s + 32],
                    xt[:, dc, :], start=dc == 0, stop=dc == DC - 1,
                    tile_position=(0, 32 * s))

            def mm_dd(b, dc):
                s = 2 * (dc % 2) + b
                nc.tensor.matmul(
                    h2[32 * s:32 * s + 32, :],
                    wall_sb[:, dc, 256 + 32 * b:256 + 32 * b + 32],
                    xt[:, dc, :], start=dc < 2, stop=dc >= DC - 2,
                    tile_position=(0, 32 * s), skip_group_check=True)

            def make_merge(row0, h2, h2sb, ddT):
                # dd halves live on psum partitions 0-63 (even dc) / 64-127
                # (odd dc); DVE lanes are partition-locked, so merge with a
                # tiny PE matmul against [I64; I64] (two 32-col tiled MMs).
                # Deferred into the next chunk's mm1 so the ACT copy it
                # depends on never stalls PE at a chunk boundary.
                def merge():
                    nc.scalar.activation(h2sb[:], h2[:], AF.Copy)
                    mps = po.tile([128, 2, 256], F32, tag="w")
                    mv = mps[:].rearrange("p a b -> p (a b)")
                    nc.tensor.matmul(mv[0:32, :], mrg_sb[:, 0:32], h2sb[:],
                                     start=True, stop=True, tile_position=(0, 0))
                    nc.tensor.matmul(mv[32:64, :], mrg_sb[:, 32:64], h2sb[:],
                                     start=True, stop=True, tile_position=(0, 32))
                    nc.scalar.activation(ddT[:], mv[0:64, :], AF.Tanh)
                    nc.scalar.dma_start(dd_d[:, row0:row0 + rc], ddT[:])
                return merge

            if last:
                # tail chunk: colgroup-major with dd/tanh finishing before
                # any of the remaining Sqrt chains (keeps the Sqrt ACT table
                # resident through both rms pairs) so the exposed tail after
                # the final matmul is just one pair's rms chain.
                for dc in range(DC):
                    for s in range(4):
                        mm_h(h0, 0, s, dc)
                    if dc == 4 and pending_merge is not None:
                        pending_merge()
                        pending_merge = None
                    if dc == 8 and pending is not None:
                        pending()
                        pending = None
                nc.scalar.activation(hT0[:], h0[:], act)
                for dc in range(DC):
                    for b in range(2):
                        mm_dd(b, dc)
                own_merge = make_merge(row0, h2, h2sb, ddT)
                for dc in range(DC):
                    for s in range(4):
                        mm_h(h1, 128, s, dc)
                    if dc == 2:
                        own_merge()
                # split gelu so pair 0's mm2 only waits on its own rows
                nc.scalar.activation(hT1[:, 0:256], h1[:, 0:256], act)
                nc.scalar.activation(hT1[:, 256:], h1[:, 256:], act)
                pk = papool.tile([128, 4, 256], F32, tag="pk")
                emit_post_pair(pk, 0, hT0, hT1)
                store_pair(pk, row0, 0)
                emit_post_pair(pk, 1, hT0, hT1)
                store_pair(pk, row0, 1)
            else:
                for dc in range(DC):
                    # rotate strip order by 2 on odd dc so the 10 blocks/dc
                    # visit the four PE col-tiles in a strict round-robin
                    # (T0T1T2T3 x5 per dc pair) -- the in-order engine queue
                    # never head-blocks on a busy tile
                    rot = 2 * (dc % 2)
                    for s0 in range(4):
                        mm_h(h0, 0, (s0 + rot) % 4, dc)
                    for s0 in range(4):
                        mm_h(h1, 128, (s0 + rot) % 4, dc)
                    for b in range(2):
                        mm_dd(b, dc)
                    if dc == 8 and pending_merge is not None:
                        pending_merge()
                        pending_merge = None
                    if dc == 12 and pending is not None:
                        pending()
                        pending = None
                    if idx == 0 and dc in (9, 17, 25):
                        # chunk 0 is paced by its own DMA stream; keep PE
                        # from crossing the HAM idle window during the
                        # supply gaps with a couple of filler matmuls.
                        for _ in range(3):
                            nc.tensor.matmul(wv[0:32, :], wz[:, 0:32], wz[:],
                                             start=True, stop=True,
                                             tile_position=(0, 0))
                nc.scalar.activation(hT0[:], h0[:], act)
                nc.scalar.activation(hT1[:], h1[:], act)
                pending_merge = make_merge(row0, h2, h2sb, ddT)
                pending = make_post(row0, hT0, hT1)
        if pending_merge is not None:
            pending_merge()
        if pending is not None:
            pending()

    nc.compile()
    return nc


def host_prep(query_vec, dw1, qkw, dd, norm_scale, nrc=NRC, rc=RC):
    """Build per-core input maps (plus shared weight arrays)."""
    rows_core = nrc * rc
    x = np.ascontiguousarray(query_vec.reshape(B * T, D)).astype(ml_dtypes.bfloat16)

    w1 = dw1[:, 0, 0, :]            # [D, 128]  pre_q
    w3 = dw1[:, 0, 2, :]            # [D, 128]  post_q
    ddp = dd[:, 0, 0:32]            # [D, 32]   pre_qdd
    ddq = dd[:, 0, 64:96]           # [D, 32]   post_qdd
    w_all = np.concatenate([w1, w3, ddp, ddq], axis=1)      # [D, 320]
    w_all = np.ascontiguousarray(w_all).astype(ml_dtypes.bfloat16)
    wall_h = np.ascontiguousarray(
        w_all.reshape(DC, 128, WCOLS).transpose(1, 0, 2))   # [128, DC, 320]

    qkw2 = np.ascontiguousarray(
        qkw[0, [0, 2]].reshape(2, 128, 128).transpose(1, 0, 2)
    ).astype(ml_dtypes.bfloat16)                             # [128, 2, 128]
    eye = np.eye(64, dtype=np.float32)
    mrg = np.concatenate([eye, eye], axis=0).astype(ml_dtypes.bfloat16)

    in_maps = []
    for c in range(NCORES):
        xc = x[c * rows_core:(c + 1) * rows_core]               # [rows, D]
        xt = np.ascontiguousarray(
            xc.reshape(nrc, rc, DC, 128).transpose(3, 0, 2, 1))  # [128,nrc,DC,rc]
        in_maps.append({"xt": xt, "wall": wall_h, "qkw2": qkw2, "mrg": mrg})
    return in_maps


_NC_CACHE = {}


def get_nc(norm_scale):
    s = float(np.asarray(norm_scale).reshape(-1)[0])
    key = (s,)
    if key not in _NC_CACHE:
        _NC_CACHE[key] = build_nc(s2_scale=1.0 / (32.0 * s * s), s2_bias=EPS / (s * s))
    return _NC_CACHE[key]


def _run_device(nc, in_maps):
    res = run_bass_kernel_spmd(nc, in_maps, list(range(NCORES)))
    wout = np.concatenate([res.results[c]["out"] for c in range(NCORES)], axis=0)
    ddo = np.concatenate([res.results[c]["ddo"] for c in range(NCORES)], axis=1)
    ddo = np.ascontiguousarray(ddo.T)                       # [B*T, 64]
    out = np.empty((B * T, WCOLS), np.float32)
    out[:, 0:128] = wout[:, 0:128]
    out[:, 128:160] = ddo[:, 0:32]
    out[:, 160:288] = wout[:, 128:256]
    out[:, 288:320] = ddo[:, 32:64]
    return out


def _run_subprocess(query_vec, dw1, qkw, dd, norm_scale):
    """Fresh-process fallback: a crashed/wedged device state lives in the
    axon client; a clean process (with core reset) usually recovers."""
    import os
    import subprocess
    import sys
    import tempfile
    d = tempfile.mkdtemp(prefix="dwp_kernel_")
    np.save(os.path.join(d, "query_vec.npy"), query_vec)
    np.save(os.path.join(d, "dw1.npy"), dw1)
    np.save(os.path.join(d, "qkw.npy"), qkw)
    np.save(os.path.join(d, "dd.npy"), dd)
    np.save(os.path.join(d, "norm_scale.npy"), norm_scale)
    prog = (
        "import numpy as np, importlib.util, sys\n"
        f"spec = importlib.util.spec_from_file_location('dwp_kernel', {__file__!r})\n"
        "m = importlib.util.module_from_spec(spec); spec.loader.exec_module(m)\n"
        f"d = {d!r}\n"
        "ins = {k: np.load(d + '/' + k + '.npy') for k in"
        " ('query_vec', 'dw1', 'qkw', 'dd', 'norm_scale')}\n"
        "out = m.kernel(_allow_subprocess=False, **ins)\n"
        "np.save(d + '/out.npy', out)\n"
    )
    env = dict(os.environ)
    env["NEURON_RT_RESET_CORES"] = "1"
    subprocess.run([sys.executable, "-c", prog], check=True, env=env,
                   timeout=1800)
    return np.load(os.path.join(d, "out.npy"))


def kernel(query_vec, dw1, qkw, dd, norm_scale, _allow_subprocess=True):
    nc = get_nc(norm_scale)
    in_maps = host_prep(query_vec, dw1, qkw, dd, norm_scale)
    try:
        out = _run_device(nc, in_maps)
    except Exception:
        if not _allow_subprocess:
            raise
        try:
            out = _run_device(nc, in_maps)       # in-process retry
        except Exception:
            out = _run_subprocess(query_vec, dw1, qkw, dd, norm_scale)
    return out.reshape(B, T, WCOLS)
